# revision 9
# baseline (speedup 1.0000x reference)
"""4-layer GraphSAGE (mean-agg SAGEConv + PReLU) on 8 Trainium2 NeuronCores.

Strategy (graph/data parallel, per sharding hint):
- Nodes are dst-sharded across 8 cores (6250/core, padded to 6272 = 49*128).
- Per core, own nodes are sorted by in-degree (SELL-style) and packed into
  128-row tiles; incoming edges become ELL slot columns, padded to a
  per-group-uniform K with pointers at a guaranteed-zero row.
- Edge-source features are fetched with dma_gather (int16 indices) from the
  full node-feature table in HBM. Since int16 is signed, the table is
  addressed through two half-views (LO = cores 0-3 rows, HI = cores 4-7).
- Slot summation runs as one strided tensor_reduce per group+half on DVE;
  mean-normalization multiplies a precomputed 1/deg broadcast.
- agg@Wl + x@Wr + b runs on the tensor engine (PE transposes to feat-major,
  bias via an appended ones-row in the stationary operand).
- PReLU on ScalarE (uniform alpha) or DVE (general alpha).
- Between layers, each core's new shard is AllGather'd so every core holds
  the full feature table for the next layer's gather.
"""

import os
import sys

import numpy as np

sys.path.insert(0, "/opt/trn_rl_repo")

import concourse.bass as bass
import concourse.bacc as bacc
import concourse.tile as tile
import concourse.mybir as mybir
from concourse.bass_utils import run_bass_kernel_spmd
from concourse.masks import make_identity

# ---- problem constants (fixed by the task) ----
N_NODES = 50000
N_EDGES = 800000
F = 64
L = 4
C = 8
OWN = N_NODES // C          # 6250
TP = 128                    # tile rows
T = (OWN + TP - 1) // TP    # 49 tiles per core
OWNP = T * TP               # 6272 padded shard rows
ROWS = C * OWNP             # 50176
HALF = ROWS // 2            # 25088
ZP = OWN                    # zero pad row, relative index inside each half

SLOT_BUDGET = 200           # max slots (lo+hi incl self) per gather group
NT_MAX = 8                  # max tiles per group (PSUM y tile = 1 bank)

_cache = {}


def _preprocess(x, edge_index):
    """Build per-core ELL gather schedules + all device input arrays."""
    src = edge_index[0].astype(np.int64)
    dst = edge_index[1].astype(np.int64)

    deg = np.bincount(dst, minlength=N_NODES).astype(np.int64)
    deginv = (1.0 / np.maximum(deg, 1)).astype(np.float32)

    # per-core degree-descending permutation of owned nodes
    perm = np.zeros((C, OWN), np.int64)   # perm[c][row] = local node id
    rank = np.zeros((C, OWN), np.int64)   # rank[c][local] = row
    for c in range(C):
        p = np.argsort(-deg[c * OWN:(c + 1) * OWN], kind="stable")
        perm[c] = p
        rank[c, p] = np.arange(OWN)

    # absolute (padded, permuted) row of any global node
    owner = src // OWN
    asrc = owner * OWNP + rank[owner, src % OWN]
    dc = dst // OWN
    drow = rank[dc, dst % OWN]          # row within owner core's shard
    half = (asrc >= HALF).astype(np.int64)   # 0 = LO source, 1 = HI

    # per-core, per-(row, half) slot positions
    # counts[c, half, row]
    cnt = np.zeros((C, 2, OWN), np.int64)
    np.add.at(cnt, (dc, half, drow), 1)

    # per-tile per-half max count, maxed across cores -> common schedule
    cnt_t = cnt.reshape(C, 2, T, TP // TP * TP // TP, -1) if False else None
    ctile = np.zeros((2, T), np.int64)
    padded = np.zeros((C, 2, OWNP), np.int64)
    padded[:, :, :OWN] = cnt
    per_tile = padded.reshape(C, 2, T, TP).max(axis=3)   # [C, 2, T]
    ctile = per_tile.max(axis=0)                          # [2, T]

    # greedy grouping: tiles -> groups with nt<=NT_MAX and
    # nt*(KL+1 + KH+1) <= SLOT_BUDGET
    groups = []  # list of (t0, nt, KLs, KHs)
    t0 = 0
    while t0 < T:
        nt = 1
        kl = ctile[0, t0] + 1
        kh = ctile[1, t0] + 1
        while t0 + nt < T and nt < NT_MAX:
            nkl = max(kl, ctile[0, t0 + nt] + 1)
            nkh = max(kh, ctile[1, t0 + nt] + 1)
            if (nt + 1) * (nkl + nkh) > SLOT_BUDGET:
                break
            kl, kh = nkl, nkh
            nt += 1
        groups.append((t0, nt, int(kl), int(kh)))
        t0 += nt

    # slot position of each edge inside its (core,row,half) bucket
    order = np.lexsort((half, drow, dc))
    g_id = ((dc * 2 + half) * OWN + drow)[order]
    first = np.ones(len(order), bool)
    first[1:] = g_id[1:] != g_id[:-1]
    group_start = np.where(first, np.arange(len(order)), 0)
    group_start = np.maximum.accumulate(group_start)
    slot = np.arange(len(order)) - group_start   # slot within bucket, sorted order
    e_slot = np.empty(N_EDGES, np.int64)
    e_slot[order] = slot

    # build per-core wrapped int16 index arrays (concatenated groups)
    idx_lo = [[] for _ in range(C)]
    idx_hi = [[] for _ in range(C)]
    tile_of = drow // TP
    p_of = drow % TP
    for c in range(C):
        sel_c = dc == c
        for (t0g, nt, kls, khs) in groups:
            arr = [np.full(nt * kls * TP, ZP, np.int64),
                   np.full(nt * khs * TP, ZP, np.int64)]
            ks = (kls, khs)
            in_g = sel_c & (tile_of >= t0g) & (tile_of < t0g + nt)
            for h in (0, 1):
                s = in_g & (half == h)
                k_in = tile_of[s] - t0g
                j = (k_in * ks[h] + e_slot[s]) * TP + p_of[s]
                rel = asrc[s] - h * HALF
                arr[h][j] = rel
                # self slot (last slot column) = own row if this core's shard
                # lives in this half, else the zero row
                if (c < 4) == (h == 0):
                    kk = np.arange(nt)
                    pp = np.arange(TP)
                    jj = ((kk[:, None] * ks[h] + (ks[h] - 1)) * TP + pp[None, :])
                    own_rel = (c * OWNP - h * HALF + (t0g + kk[:, None]) * TP
                               + pp[None, :])
                    arr[h][jj.ravel()] = own_rel.ravel()
            idx_lo[c].append(arr[0])
            idx_hi[c].append(arr[1])

    def wrap(parts):
        flat = np.concatenate(parts)
        a = flat.reshape(-1, 16).T.astype(np.int16)   # [16, n/16]
        return np.ascontiguousarray(np.tile(a, (8, 1)))  # [128, n/16]

    idx_lo = [wrap(v) for v in idx_lo]
    idx_hi = [wrap(v) for v in idx_hi]

    # per-core deginv broadcast [128, T*F] (row-order, pad rows -> 0)
    dgi = np.zeros((C, 128, T * F), np.float32)
    for c in range(C):
        dr = np.zeros(OWNP, np.float32)
        dr[:OWN] = deginv[c * OWN + perm[c]]
        dgi[c] = np.repeat(dr.reshape(T, TP).T, F, axis=1)

    # permuted padded input features, shared by all cores
    x0g = np.zeros((ROWS, F), np.float32)
    for c in range(C):
        x0g[c * OWNP:c * OWNP + OWN] = x[c * OWN + perm[c]]

    return dict(groups=groups, idx_lo=idx_lo, idx_hi=idx_hi, dgi=dgi,
                x0g=x0g, perm=perm)


def _build(groups, cl, ch, uniform_alpha):
    """Build the SPMD Bass program. cl/ch = idx tensor column counts."""
    nc = bacc.Bacc(None, target_bir_lowering=False, num_devices=C)
    f32 = mybir.dt.float32

    x0g = nc.dram_tensor("x0g", [ROWS, F], f32, kind="ExternalInput")
    idx_lo = nc.dram_tensor("idx_lo", [128, cl], mybir.dt.int16, kind="ExternalInput")
    idx_hi = nc.dram_tensor("idx_hi", [128, ch], mybir.dt.int16, kind="ExternalInput")
    dgi = nc.dram_tensor("dgi", [128, T * F], f32, kind="ExternalInput")
    wls = nc.dram_tensor("wls", [F, L * F], f32, kind="ExternalInput")
    wrs = nc.dram_tensor("wrs", [F + 1, L * F], f32, kind="ExternalInput")  # +bias row
    abt = nc.dram_tensor("abt", [128, L * NT_MAX * F], f32, kind="ExternalInput")
    rmask = nc.dram_tensor("rmask", [128, 1], f32, kind="ExternalInput")
    y_out = nc.dram_tensor("y_out", [OWNP, F], f32, kind="ExternalOutput")

    shard = [nc.dram_tensor(f"shard{l}", [OWNP, F], f32, kind="Internal")
             for l in range(L - 1)]
    gath = [nc.dram_tensor(f"gath{l}", [ROWS, F], f32, kind="Internal",
                           addr_space="Shared") for l in range(L - 1)]

    with tile.TileContext(nc) as tc:
        with (
            tc.tile_pool(name="const", bufs=1) as constp,
            tc.tile_pool(name="glo", bufs=2) as glop,
            tc.tile_pool(name="ghi", bufs=2) as ghip,
            tc.tile_pool(name="work", bufs=2) as workp,
            tc.tile_pool(name="tsbp", bufs=2) as tsbp,
            tc.tile_pool(name="outp", bufs=2) as outp,
            tc.tile_pool(name="ptp", bufs=1, space="PSUM") as ptp,
            tc.tile_pool(name="ypp", bufs=2, space="PSUM") as ypp,
        ):
            # ---- preamble: persistent SBUF state ----
            il_sb = constp.tile([128, cl], mybir.dt.int16)
            nc.sync.dma_start(il_sb[:], idx_lo[:])
            ih_sb = constp.tile([128, ch], mybir.dt.int16)
            nc.sync.dma_start(ih_sb[:], idx_hi[:])
            dgi_sb = constp.tile([128, T * F], f32)
            nc.sync.dma_start(dgi_sb[:], dgi[:])
            wl_sb = constp.tile([F, L * F], f32)
            nc.sync.dma_start(wl_sb[:], wls[:])
            wr_sb = constp.tile([F + 1, L * F], f32)
            nc.sync.dma_start(wr_sb[:], wrs[:])
            ab_sb = None
            if not uniform_alpha:
                ab_sb = constp.tile([128, L * NT_MAX * F], f32)
                nc.sync.dma_start(ab_sb[:], abt[:])
            rm_sb = constp.tile([128, 1], f32)
            nc.sync.dma_start(rm_sb[:], rmask[:])
            ident = constp.tile([128, 128], f32)
            make_identity(nc, ident[:])

            for l in range(L):
                src = x0g if l == 0 else gath[l - 1]
                src_lo = src[0:HALF, :]
                src_hi = src[HALF:2 * HALF, :]
                col_l = 0
                col_h = 0
                for (t0g, nt, kls, khs) in groups:
                    nl = nt * kls * TP     # num lo idxs
                    nh = nt * khs * TP
                    g_lo = glop.tile([128, nt * kls * F], f32, tag="glo")
                    nc.gpsimd.dma_gather(
                        g_lo[:].rearrange("p (k f) -> p k f", f=F),
                        src_lo, il_sb[:, col_l:col_l + nl // 16],
                        num_idxs=nl, num_idxs_reg=nl, elem_size=F,
                        single_packet=False,
                    )
                    g_hi = ghip.tile([128, nt * khs * F], f32, tag="ghi")
                    nc.gpsimd.dma_gather(
                        g_hi[:].rearrange("p (k f) -> p k f", f=F),
                        src_hi, ih_sb[:, col_h:col_h + nh // 16],
                        num_idxs=nh, num_idxs_reg=nh, elem_size=F,
                        single_packet=False,
                    )
                    col_l += nl // 16
                    col_h += nh // 16

                    # slot sums (excluding the self slot = last column)
                    agg = workp.tile([128, nt * F], f32, tag="agg")
                    tmp = workp.tile([128, nt * F], f32, tag="tmp")
                    nc.vector.tensor_reduce(
                        agg[:],
                        g_lo[:].rearrange("p (t k f) -> p t f k", t=nt, f=F)[
                            :, :, :, 0:kls - 1],
                        axis=mybir.AxisListType.X, op=mybir.AluOpType.add,
                    )
                    nc.vector.tensor_reduce(
                        tmp[:],
                        g_hi[:].rearrange("p (t k f) -> p t f k", t=nt, f=F)[
                            :, :, :, 0:khs - 1],
                        axis=mybir.AxisListType.X, op=mybir.AluOpType.add,
                    )
                    nc.vector.tensor_add(agg[:], agg[:], tmp[:])
                    nc.vector.tensor_mul(
                        agg[:], agg[:], dgi_sb[:, t0g * F:(t0g + nt) * F])

                    # PE transposes: aggT then xtT (= lo_self + hi_self)
                    pt = ptp.tile([64, 2 * nt * 128], f32, space="PSUM", tag="pt")
                    for k in range(nt):
                        nc.tensor.matmul(
                            pt[0:64, k * 128:(k + 1) * 128],
                            agg[:, k * F:(k + 1) * F], ident[:],
                            is_transpose=True, start=True, stop=True)
                        ca = (nt + k) * 128
                        nc.tensor.matmul(
                            pt[0:64, ca:ca + 128],
                            g_lo[:, ((k + 1) * kls - 1) * F:((k + 1) * kls) * F],
                            ident[:], is_transpose=True, start=True, stop=False)
                        nc.tensor.matmul(
                            pt[0:64, ca:ca + 128],
                            g_hi[:, ((k + 1) * khs - 1) * F:((k + 1) * khs) * F],
                            ident[:], is_transpose=True, start=False, stop=True)

                    tsb = tsbp.tile([65, 2 * nt * 128], f32, tag="tsb")
                    nc.vector.tensor_copy(tsb[0:64, :], pt[:])
                    nc.vector.memset(tsb[64:65, nt * 128:2 * nt * 128], 1.0)

                    # matmuls: y = aggT.T @ Wl + [xtT;1].T @ [Wr;b]
                    y_ps = ypp.tile([128, nt * F], f32, space="PSUM", tag="y")
                    for k in range(nt):
                        nc.tensor.matmul(
                            y_ps[:, k * F:(k + 1) * F],
                            tsb[0:64, k * 128:(k + 1) * 128],
                            wl_sb[:, l * F:(l + 1) * F],
                            start=True, stop=False)
                        nc.tensor.matmul(
                            y_ps[:, k * F:(k + 1) * F],
                            tsb[0:65, (nt + k) * 128:(nt + k + 1) * 128],
                            wr_sb[:, l * F:(l + 1) * F],
                            start=False, stop=True)

                    # PReLU epilogue
                    y_sb = outp.tile([128, nt * F], f32, tag="ysb")
                    if uniform_alpha is not None and uniform_alpha is not False:
                        nc.scalar.activation(
                            y_sb[:], y_ps[:], mybir.ActivationFunctionType.Prelu,
                            alpha=float(uniform_alpha))
                    else:
                        t1 = outp.tile([128, nt * F], f32, tag="t1")
                        nc.vector.tensor_scalar_min(t1[:], y_ps[:], 0.0)
                        nc.vector.tensor_mul(
                            t1[:], t1[:],
                            ab_sb[:, l * NT_MAX * F:l * NT_MAX * F + nt * F])
                        nc.scalar.activation(
                            y_sb[:], y_ps[:], mybir.ActivationFunctionType.Relu)
                        nc.vector.tensor_add(y_sb[:], y_sb[:], t1[:])

                    last_group = t0g + nt == T
                    if l < L - 1 and last_group:
                        # zero the shard's pad rows (guaranteed-zero gather rows)
                        nc.vector.tensor_scalar_mul(
                            y_sb[:, (nt - 1) * F:nt * F],
                            y_sb[:, (nt - 1) * F:nt * F], rm_sb[:, 0:1])

                    dest = y_out if l == L - 1 else shard[l]
                    nc.sync.dma_start(
                        dest[t0g * TP:(t0g + nt) * TP, :].rearrange(
                            "(k p) f -> p k f", p=TP),
                        y_sb[:].rearrange("p (k f) -> p k f", f=F))

                if l < L - 1:
                    nc.gpsimd.collective_compute(
                        "AllGather", mybir.AluOpType.bypass,
                        replica_groups=[list(range(C))],
                        ins=[shard[l][:]], outs=[gath[l][:]],
                    )
    nc.compile()
    return nc


def _prepare(x, edge_index, Wl, Wr, b, alpha):
    x = np.asarray(x, np.float32)
    edge_index = np.asarray(edge_index, np.int32)
    Wl = np.asarray(Wl, np.float32)
    Wr = np.asarray(Wr, np.float32)
    b = np.asarray(b, np.float32)
    alpha = np.asarray(alpha, np.float32)

    key = (edge_index.tobytes(), alpha.tobytes())
    import hashlib
    key = hashlib.sha1(key[0] + key[1]).hexdigest()
    if key not in _cache:
        pre = _preprocess(x, edge_index)
        ua = False  # Prelu ACT path unsupported by sim/lowering; use DVE path
        cl = pre["idx_lo"][0].shape[1]
        ch = pre["idx_hi"][0].shape[1]
        nc = _build(pre["groups"], cl, ch, ua)
        _cache.clear()
        _cache[key] = (pre, nc, ua)
    return _cache[key]


def _in_maps(pre, ua, x, Wl, Wr, b, alpha):
    # x0g depends on x; rebuild it (cheap) in case x changed under same graph
    perm = pre["perm"]
    x0g = np.zeros((ROWS, F), np.float32)
    for c in range(C):
        x0g[c * OWNP:c * OWNP + OWN] = x[c * OWN + perm[c]]

    wls = np.concatenate([Wl[l] for l in range(L)], axis=1).astype(np.float32)
    wrs = np.concatenate(
        [np.vstack([Wr[l], b[l][None, :]]) for l in range(L)], axis=1
    ).astype(np.float32)
    abt = np.concatenate(
        [np.tile(alpha[l][None, :], (128, NT_MAX)) for l in range(L)], axis=1
    ).astype(np.float32)
    rmask_arr = np.ones((128, 1), np.float32)
    rmask_arr[OWN - (T - 1) * TP:, 0] = 0.0

    maps = []
    for c in range(C):
        maps.append({
            "x0g": x0g,
            "idx_lo": pre["idx_lo"][c],
            "idx_hi": pre["idx_hi"][c],
            "dgi": np.ascontiguousarray(pre["dgi"][c]),
            "wls": wls,
            "wrs": wrs,
            "abt": abt,
            "rmask": rmask_arr,
        })
    return maps


def _assemble(pre, results):
    perm = pre["perm"]
    out = np.zeros((N_NODES, F), np.float32)
    for c in range(C):
        y = results[c]["y_out"]
        out[c * OWN + perm[c]] = y[:OWN]
    return out


def _run(inputs, trace=False):
    x = np.asarray(inputs["x"], np.float32)
    edge_index = np.asarray(inputs["edge_index"], np.int32)
    Wl = np.asarray(inputs["Wl"], np.float32)
    Wr = np.asarray(inputs["Wr"], np.float32)
    b = np.asarray(inputs["b"], np.float32)
    alpha = np.asarray(inputs["alpha"], np.float32)

    pre, nc, ua = _prepare(x, edge_index, Wl, Wr, b, alpha)
    maps = _in_maps(pre, ua, x, Wl, Wr, b, alpha)
    if trace:
        _install_ntff_hook()
    bkr = run_bass_kernel_spmd(nc, maps, core_ids=list(range(C)), trace=trace)
    return _assemble(pre, bkr.results), bkr


def _install_ntff_hook():
    """Provide antenv.axon_hooks (absent in this image) for trace=True."""
    import types

    if "antenv.axon_hooks" in sys.modules:
        return
    try:
        from trn_agent_boot.trn_boot import _ntff_profile_via_ctypes
        hook = _ntff_profile_via_ctypes("/opt/axon/libaxon_pjrt.so")
    except Exception:
        hook = None
    mod = types.ModuleType("antenv.axon_hooks")
    state = {"hook": hook}
    mod.set_axon_ntff_profile_hook = lambda h: state.update(hook=h)
    mod.get_axon_ntff_profile_hook = lambda: state["hook"]
    sys.modules["antenv.axon_hooks"] = mod
    import antenv
    antenv.axon_hooks = mod


def kernel(**inputs) -> np.ndarray:
    out, _ = _run(inputs, trace=False)
    return out
